# revision 26
# baseline (speedup 1.0000x reference)
"""Trainium2 Bass kernel for BlockFFTDirectPrior.

Computes out = irfft(einsum('bjn,ijn->bin', rfft(x_blocks), conj(W)))
reshaped to [B, 4096], for x [4096, 4096] f32, W [16, 16, 129] complex
(block size 256).

Strategy: data-parallel over the batch axis across 8 NeuronCores (512 rows
each). Per core, the 512 rows are processed as two 256-row slabs flowing
through a 4-stage PE pipeline so input DMA, compute, the two partition
regroups, and output stores all overlap:

  T: transpose x tiles (PE transpose vs identity)     -> xt [t, dc, b] fp16
  F: real DFT as fp16 matmuls (contract t)            -> xf [n, j, ri, b]
  E: per-frequency 16x16 complex mixing as 8-frequency
     block-diagonal fp16 matmuls (K = (f,j) = 128)    -> yy [n', g, ri, b]
  I: real inverse DFT, data stationary (fp16 weights,
     FWL), which restores [b, m] orientation for free -> out rows

DFT/IDFT row order is swizzled to r = f*16+g so the two partition
regroups between F/E and E/I become per-g (resp. per-i) affine
SBUF->SBUF DMAs that carry both the real and imag halves in one
transfer. Regroups and stores ride the two fast HWDGE rings (sync +
scalar) ordered to match the pipeline; intermediates are fp16, which
halves regroup bytes and doubles LDWEIGHTS rate (FWL). Accumulation
groups are bank-interleaved in PSUM so one LDWEIGHTS feeds two matmuls.
"""

import os
import numpy as np
from contextlib import ExitStack

import concourse.bass as bass
import concourse.tile as tile
from concourse import bacc, mybir
from concourse.bass_utils import run_bass_kernel_spmd

NCORES = 8
B_FULL, D_IN, D_OUT, BS = 4096, 4096, 4096, 256
BC = B_FULL // NCORES          # 512 batch rows per core
SLAB = 128                     # rows per pipeline slab (4 slabs per core)
KIN = KOUT = 16
NG = 16                        # groups of 8 frequencies covering n=0..127
F16 = mybir.dt.float16
F32 = mybir.dt.float32

_CACHE = {}
LAST_RESULTS = None            # BassKernelResults of the most recent run


# DFT row swizzle: row r holds frequency n = 16*((r%32)//4) + 4*(r//32) +
# (r%4).  Group g = n%16 then occupies rows {32*(g//4) + g%4 + 4k}, a
# stride-4 partition slice: its 8 partitions map to 8 distinct SBUF AXI
# ports (port = 2*((p%32)//4) + p//64), twice the read bandwidth of a
# stride-16 pattern.  The same stride-4 property holds for the E-output
# rows of each i, and the regrouped yh rows come out in natural frequency
# order (row p holds n = p), so the IDFT matrix needs no permutation.
PERM = np.array([16 * ((r % 32) // 4) + 4 * (r // 32) + (r % 4)
                 for r in range(128)])


def _grp_rows(g):
    # first row of the stride-4 slice holding group/output index g
    return 32 * (g // 4) + (g % 4)


def _build_consts(W_real, W_imag):
    """Constant matrices in the exact SBUF layouts the kernel reads."""
    f16 = np.float16
    t = np.arange(BS)
    n0 = np.arange(128)
    ang = 2.0 * np.pi / BS

    CF0 = np.cos(ang * np.outer(t, n0))
    CF1 = np.empty((BS, 128))
    CF1[:, 0] = np.cos(np.pi * t)
    p = np.arange(1, 128)
    CF1[:, 1:] = -np.sin(ang * np.outer(t, p))
    CF0 = CF0[:, PERM]
    CF1 = CF1[:, PERM]
    cfs = np.stack([
        np.concatenate([CF0[:128], CF0[128:]], axis=1),
        np.concatenate([CF1[:128], CF1[128:]], axis=1),
    ], axis=1).astype(f16)                                  # [128, 2, 256]

    # wpk[(k*16+j), g, c, r'(i,k)] = M_c[i, j, 16k+g];  M = (Wr, Wi, -Wi);
    # r'(i,k) = 32*(i//4) + i%4 + 4k is the E-output row for (i, k)
    wpk = np.zeros((128, NG, 3, 128), dtype=f16)
    jj = np.arange(KIN)[:, None, None]
    ii = np.arange(KOUT)[None, :, None]
    kk = np.arange(8)[None, None, :]
    rr = 32 * (ii // 4) + ii % 4 + 4 * kk
    for g in range(NG):
        for c, M in enumerate((W_real, W_imag, -W_imag)):
            wpk[kk * 16 + jj, g, c, rr] = M[ii, jj, 16 * kk + g]
    # wnyq[j, r'(i,0)] = Wr[i, j, 128]: the Nyquist matmul output lands
    # directly on the E-output rows for k=0, 32-aligned for the copies
    wnyq = np.zeros((KIN, 128), dtype=f16)
    for i in range(KOUT):
        wnyq[:, 32 * (i // 4) + i % 4] = W_real[i, :, 128]

    # IDFT matrices in natural frequency row order (yh row p holds n = p)
    m = np.arange(BS)
    D0 = np.empty((128, BS))
    D0[0] = 1.0 / BS
    nn = np.arange(1, 128)
    D0[1:] = (2.0 / BS) * np.cos(ang * np.outer(nn, m))
    D1 = np.empty((128, BS))
    D1[0] = ((-1.0) ** m) / BS
    D1[1:] = -(2.0 / BS) * np.sin(ang * np.outer(nn, m))
    dmat = np.stack([D0, D1], axis=1).astype(f16)  # [128, 2, 256]

    ident = np.eye(128, dtype=np.float32)
    return {"cfs": cfs, "wpk": wpk, "wnyq": wnyq, "dmat": dmat, "ident": ident}


def _build_program():
    nc = bacc.Bacc(
        "TRN2", target_bir_lowering=False, debug=False, num_devices=NCORES
    )
    x_d = nc.dram_tensor("x", [BC, D_IN], F32, kind="ExternalInput").ap()
    cfs_d = nc.dram_tensor("cfs", [128, 2, 256], F16, kind="ExternalInput").ap()
    wpk_d = nc.dram_tensor("wpk", [128, NG, 3, 128], F16, kind="ExternalInput").ap()
    wnyq_d = nc.dram_tensor("wnyq", [KIN, 128], F16, kind="ExternalInput").ap()
    dmat_d = nc.dram_tensor("dmat", [128, 2, 256], F16, kind="ExternalInput").ap()
    ident_d = nc.dram_tensor("ident", [128, 128], F32, kind="ExternalInput").ap()
    out_d = nc.dram_tensor("out", [BC, D_OUT], F32, kind="ExternalOutput").ap()

    cp_state = [0]

    with tile.TileContext(nc) as tc, ExitStack() as ctx:
        def copy(dst, src):
            # alternate PSUM->SBUF copies between DVE and ACT
            if cp_state[0] % 2 == 0:
                nc.vector.tensor_copy(dst, src)
            else:
                nc.scalar.copy(dst, src)
            cp_state[0] += 1

        consts = ctx.enter_context(tc.tile_pool(name="consts", bufs=1))
        xsp = ctx.enter_context(tc.tile_pool(name="xsp", bufs=3))
        xtp = ctx.enter_context(tc.tile_pool(name="xtp", bufs=1))
        mid1 = ctx.enter_context(tc.tile_pool(name="mid1", bufs=1))
        mid2 = ctx.enter_context(tc.tile_pool(name="mid2", bufs=2))
        osp = ctx.enter_context(tc.tile_pool(name="osp", bufs=2))
        ps = ctx.enter_context(tc.tile_pool(name="ps", bufs=4, space="PSUM"))

        cfs = consts.tile([128, 2, 256], F16)
        wpk = consts.tile([128, NG, 3, 128], F16)
        wnyq = consts.tile([KIN, 128], F16)
        dmat = consts.tile([128, 2, 256], F16)
        ident = consts.tile([128, 128], F32)

        # ident/cfs (small, needed first) go ahead of x on the sync ring
        # (the scalar ring gets starved by packet round-robin early on);
        # bulky-but-late consts ride the gpsimd (SWDGE) ring
        nc.sync.dma_start(ident[:], ident_d)
        nc.sync.dma_start(cfs[:], cfs_d)
        nc.gpsimd.dma_start(wpk[:], wpk_d)
        nc.gpsimd.dma_start(dmat[:], dmat_d)
        nc.gpsimd.dma_start(wnyq[:], wnyq_d)

        # ---- input loads: all on the sync ring, in order.  One ring's
        # engines drain its DMAs in issue order, so chunk 0 completes
        # first (~12us) instead of fair-sharing with later chunks.
        xs = [xsp.tile([128, D_IN], F32, tag="xs", name=f"xs{i}")
              for i in range(4)]
        for bc in range(4):
            for h in range(2):
                nc.sync.dma_start(
                    xs[bc][:, 2048 * h:2048 * (h + 1)],
                    x_d[128 * bc:128 * (bc + 1), 2048 * h:2048 * (h + 1)])

        # per-slab tiles (4 slabs of 128 rows); double-buffered so one
        # slab's regroup DMAs never wait on the previous slab's consumers
        xt = [xtp.tile([128, 32, SLAB], F16, tag="xt", name=f"xt{i}")
              for i in range(4)]
        xf = [mid1.tile([128, KIN, 2, SLAB], F16, tag="xf", name=f"xf{i}")
              for i in range(4)]
        # gg/yh are split into halves so E/I can start once the first half
        # of a regroup has landed instead of waiting for all 16 DMAs
        gg = [[mid2.tile([128, NG // 2, 2, SLAB], F16, tag=f"gg{h}",
                         name=f"gg{i}_{h}") for h in range(2)]
              for i in range(4)]
        yy = [mid1.tile([128, NG, 2, SLAB], F16, tag="yy", name=f"yy{i}")
              for i in range(4)]
        yh = [[mid2.tile([128, KOUT // 2, 2, SLAB], F16, tag=f"yh{h}",
                         name=f"yh{i}_{h}") for h in range(2)]
              for i in range(4)]
        gnyq = [consts.tile([KIN, SLAB], F16, tag="gnyq", name=f"gnyq{i}")
                for i in range(4)]

        # regroup ring plans: regroup1(s0) avoids sync (still loading x);
        # later regroups spread across all three DGE units
        RR_NOSYNC = [nc.gpsimd, nc.scalar] * 8
        RR_ALL = [nc.gpsimd, nc.sync, nc.scalar] * 5 + [nc.gpsimd]

        def stage_T(s):
            # transpose chunk s of x into xt[s]
            for dcg in range(4):
                pt = ps.tile([128, 8, 128], F32, tag="ps")
                for q in range(8):
                    dc = dcg * 8 + q
                    nc.tensor.transpose(
                        pt[:, q, :], xs[s][:, 128 * dc:128 * (dc + 1)],
                        ident[:],
                    )
                copy(xt[s][:, 8 * dcg:8 * dcg + 8, :], pt[:])

        def stage_F(s):
            # real DFT: xf[s][n, j, which, b] = sum_t cfs[t, which, n] xt[t, (j,tc), b]
            # j pairs map to the two banks of one PSUM slot (slots 0 / 2) so
            # accumulation groups never interleave within a bank, while each
            # LDWEIGHTS (cfs half) feeds two matmuls.
            # slot map: j0->0(bank0), j1->2(bank1), then j2->1(bank0), j3->3
            # (a bank's second group starts only after its first stopped)
            for which in range(2):
                for jg in range(4):
                    pf = ps.tile([128, 4, 256], F32, tag="ps")
                    for half in range(2):
                        for tc_ in range(2):
                            for bank in range(2):
                                j = 4 * jg + 2 * half + bank
                                nc.tensor.matmul(
                                    pf[:, 2 * bank + half, 0:SLAB],
                                    cfs[:, which, 128 * tc_:128 * (tc_ + 1)],
                                    xt[s][:, 2 * j + tc_, :],
                                    start=(tc_ == 0),
                                    stop=(tc_ == 1),
                                )
                    # slots (0,2,1,3) hold j order (0,1,2,3)
                    copy(xf[s][:, 4 * jg:4 * jg + 4, which, :],
                         pf[:, :, 0:SLAB].rearrange(
                             "p (a b) n -> p b a n", a=2))

        def regroup1(s, rings):
            # gg[s][(k,j), g, ri, b] = xf[s][r(k,g), j, ri, b]; one DMA per
            # g reading a stride-4 partition slice (8 SBUF ports)
            for g in range(NG):
                a = _grp_rows(g)
                rings[g % len(rings)].dma_start(
                    out=gg[s][g // 8][:, g % 8, :, :],
                    in_=xf[s][a:a + 29:4, :, :, :])

        def stage_E(s):
            # per-frequency-group complex mixing, two g per PSUM slot:
            # g even -> slots 0 (Yr) / 2 (Yi), g odd -> slots 1 / 3
            for gp in range(8):
                pe = ps.tile([128, 4, 256], F32, tag="ps")
                for half in range(2):
                    g = 2 * gp + half
                    nc.tensor.matmul(pe[:, half, 0:SLAB], wpk[:, g, 0, :],
                                     gg[s][g // 8][:, g % 8, 0, :],
                                     start=True, stop=False)
                    nc.tensor.matmul(pe[:, 2 + half, 0:SLAB], wpk[:, g, 0, :],
                                     gg[s][g // 8][:, g % 8, 1, :],
                                     start=True, stop=False)
                    nc.tensor.matmul(pe[:, half, 0:SLAB], wpk[:, g, 1, :],
                                     gg[s][g // 8][:, g % 8, 1, :],
                                     start=False, stop=True)
                    nc.tensor.matmul(pe[:, 2 + half, 0:SLAB], wpk[:, g, 2, :],
                                     gg[s][g // 8][:, g % 8, 0, :],
                                     start=False, stop=True)
                # slots (0,2,1,3) hold (g0 Yr, g0 Yi, g1 Yr, g1 Yi)
                copy(yy[s][:, 2 * gp:2 * gp + 2, :, :],
                     pe[:, :, 0:SLAB].rearrange(
                         "p (a b) n -> p b a n", a=2))
            # Nyquist einsum lands in the (f=0,g=0) rows of the imag half
            # (the otherwise meaningless Zi[0] slots); regroup2 then routes
            # it to yh[.,1,...] row 0, where dmat row 0 of D1 applies it.
            pyn = ps.tile([128, 256], F32, tag="ps")
            nc.tensor.matmul(pyn[:, 0:SLAB], wnyq[:], gnyq[s][:],
                             start=True, stop=True)
            # Zi[0] rows for i = 4a+c sit at partition 32a+c; pyn rows
            # match, so each copy reads/writes a 32-aligned partition base
            for a in range(4):
                copy(yy[s][32 * a:32 * a + 4, 0, 1, :],
                     pyn[32 * a:32 * a + 4, 0:SLAB])

        def regroup2(s, rings):
            # yh[s][n, i, ri, b] = yy[s][r'(i,k), g, ri, b]; one DMA per i
            # reading a stride-4 partition slice; yh rows come out in
            # natural frequency order n = 16k+g
            for i in range(KOUT):
                a = _grp_rows(i)
                rings[i % len(rings)].dma_start(
                    out=yh[s][i // 8][:, i % 8, :, :],
                    in_=yy[s][a:a + 29:4, :, :, :])

        def stage_I(s):
            # inverse DFT with the data stationary -> [b, m] orientation;
            # four i per PSUM slot, groups sequential within each bank
            for ig in range(4):
                osb = osp.tile([128, 1024], F32, tag="os")
                po = ps.tile([128, 4, 256], F32, tag="ps")
                for iq in range(4):
                    i = 4 * ig + iq
                    nc.tensor.matmul(
                        po[:, iq, :], yh[s][i // 8][:, i % 8, 0, :],
                        dmat[:, 0, :], start=True, stop=False)
                    nc.tensor.matmul(
                        po[:, iq, :], yh[s][i // 8][:, i % 8, 1, :],
                        dmat[:, 1, :], start=False, stop=True)
                copy(osb[:], po[:])
                eng = nc.sync if (s + ig) % 2 == 0 else nc.scalar
                eng.dma_start(
                    out_d[SLAB * s:SLAB * (s + 1),
                          1024 * ig:1024 * (ig + 1)],
                    osb[:],
                )

        # ---- pipelined emission (per-engine queues in execution order)
        # PE warmup: dummy transposes of ident bridge the input-load
        # window so HAM unthrottles before the real work starts
        for w in range(4):
            pw = ps.tile([128, 4, 128], F32, tag="ps", name=f"pw{w}")
            for q in range(4):
                nc.tensor.transpose(pw[:, q, :], ident[:], ident[:])

        def front(s, rings):
            stage_T(s)
            stage_F(s)
            regroup1(s, rings)
            nc.gpsimd.dma_start(out=gnyq[s][:], in_=xf[s][0:1, :, 1, :])

        front(0, RR_NOSYNC)
        front(1, RR_NOSYNC)
        stage_E(0)
        regroup2(0, RR_ALL)
        front(2, RR_ALL)
        stage_E(1)
        regroup2(1, RR_ALL)
        stage_I(0)
        front(3, RR_ALL)
        stage_E(2)
        regroup2(2, RR_ALL)
        stage_I(1)
        stage_E(3)
        regroup2(3, RR_ALL)
        stage_I(2)
        stage_I(3)

    nc.compile()
    return nc


def _get_program():
    if "nc" not in _CACHE:
        _CACHE["nc"] = _build_program()
    return _CACHE["nc"]


def _install_ntff_hook():
    """Provide antenv.axon_hooks (absent in this image) so that
    run_bass_kernel_spmd(trace=True) can capture NTFF profiles through the
    axon client library."""
    import sys
    import types
    import ctypes
    import contextlib

    if "antenv.axon_hooks" in sys.modules:
        return
    try:
        lib = ctypes.CDLL("/opt/axon/libaxon_pjrt.so")
    except OSError:
        return
    if not hasattr(lib, "axon_start_nrt_profile"):
        return
    lib.axon_start_nrt_profile.argtypes = [
        ctypes.POINTER(ctypes.c_int64),
        ctypes.c_size_t,
    ]
    lib.axon_start_nrt_profile.restype = ctypes.c_int64
    lib.axon_stop_nrt_profile.argtypes = [ctypes.c_char_p]
    lib.axon_stop_nrt_profile.restype = ctypes.c_int64

    @contextlib.contextmanager
    def _hook(output_dir, device_ids):
        import jax

        jax.devices()
        if device_ids:
            ids = (ctypes.c_int64 * len(device_ids))(*device_ids)
            rc = lib.axon_start_nrt_profile(ids, len(device_ids))
        else:
            rc = lib.axon_start_nrt_profile(None, 0)
        if rc != 0:
            raise RuntimeError(f"axon_start_nrt_profile rc={rc}")
        try:
            yield
        finally:
            n = lib.axon_stop_nrt_profile(str(output_dir).encode())
            print(f"ntff profile: {n} file(s) -> {output_dir}")

    mod = types.ModuleType("antenv.axon_hooks")
    state = {"hook": _hook}
    mod.get_axon_ntff_profile_hook = lambda: state["hook"]
    mod.set_axon_ntff_profile_hook = lambda h: state.update(hook=h)
    sys.modules["antenv.axon_hooks"] = mod
    import antenv

    antenv.axon_hooks = mod


def kernel(x, W_real, W_imag, block_size, out_features):
    global LAST_RESULTS
    x = np.ascontiguousarray(np.asarray(x, dtype=np.float32))
    Wr = np.asarray(W_real, dtype=np.float32)
    Wi = np.asarray(W_imag, dtype=np.float32)
    assert int(block_size) == BS and int(out_features) == D_OUT
    assert x.shape == (B_FULL, D_IN) and Wr.shape == (KOUT, KIN, 129)

    nc = _get_program()
    consts = _build_consts(Wr, Wi)
    core_ids = list(range(NCORES))
    in_maps = [
        {"x": np.ascontiguousarray(x[c * BC:(c + 1) * BC]), **consts}
        for c in core_ids
    ]
    trace = bool(int(os.environ.get("KERNEL_TRACE", "0")))
    if trace:
        _install_ntff_hook()
    res = run_bass_kernel_spmd(nc, in_maps, core_ids, trace=trace)
    LAST_RESULTS = res
    out = np.concatenate([res.results[c]["out"] for c in core_ids], axis=0)
    return np.ascontiguousarray(out.astype(np.float32))


# revision 27
# speedup vs baseline: 1.1125x; 1.1125x over previous
"""Trainium2 Bass kernel for BlockFFTDirectPrior.

Computes out = irfft(einsum('bjn,ijn->bin', rfft(x_blocks), conj(W)))
reshaped to [B, 4096], for x [4096, 4096] f32, W [16, 16, 129] complex
(block size 256).

Strategy: data-parallel over the batch axis across 8 NeuronCores (512 rows
each). Per core, the 512 rows are processed as two 256-row slabs flowing
through a 4-stage PE pipeline so input DMA, compute, the two partition
regroups, and output stores all overlap:

  T: transpose x tiles (PE transpose vs identity)     -> xt [t, dc, b] fp16
  F: real DFT as fp16 matmuls (contract t)            -> xf [n, j, ri, b]
  E: per-frequency 16x16 complex mixing as 8-frequency
     block-diagonal fp16 matmuls (K = (f,j) = 128)    -> yy [n', g, ri, b]
  I: real inverse DFT, data stationary (fp16 weights,
     FWL), which restores [b, m] orientation for free -> out rows

DFT/IDFT row order is swizzled to r = f*16+g so the two partition
regroups between F/E and E/I become per-g (resp. per-i) affine
SBUF->SBUF DMAs that carry both the real and imag halves in one
transfer. Regroups and stores ride the two fast HWDGE rings (sync +
scalar) ordered to match the pipeline; intermediates are fp16, which
halves regroup bytes and doubles LDWEIGHTS rate (FWL). Accumulation
groups are bank-interleaved in PSUM so one LDWEIGHTS feeds two matmuls.
"""

import os
import numpy as np
from contextlib import ExitStack

import concourse.bass as bass
import concourse.tile as tile
from concourse import bacc, mybir
from concourse.bass_utils import run_bass_kernel_spmd

NCORES = 8
B_FULL, D_IN, D_OUT, BS = 4096, 4096, 4096, 256
BC = B_FULL // NCORES          # 512 batch rows per core
SLAB = 256                     # rows per pipeline slab (2 slabs per core)
KIN = KOUT = 16
NG = 16                        # groups of 8 frequencies covering n=0..127
F16 = mybir.dt.float16
F32 = mybir.dt.float32

_CACHE = {}
LAST_RESULTS = None            # BassKernelResults of the most recent run


# DFT row swizzle: row r holds frequency n = 16*((r%32)//4) + 4*(r//32) +
# (r%4).  Group g = n%16 then occupies rows {32*(g//4) + g%4 + 4k}, a
# stride-4 partition slice: its 8 partitions map to 8 distinct SBUF AXI
# ports (port = 2*((p%32)//4) + p//64), twice the read bandwidth of a
# stride-16 pattern.  The same stride-4 property holds for the E-output
# rows of each i, and the regrouped yh rows come out in natural frequency
# order (row p holds n = p), so the IDFT matrix needs no permutation.
PERM = np.array([16 * ((r % 32) // 4) + 4 * (r // 32) + (r % 4)
                 for r in range(128)])


def _grp_rows(g):
    # first row of the stride-4 slice holding group/output index g
    return 32 * (g // 4) + (g % 4)


def _build_consts(W_real, W_imag):
    """Constant matrices in the exact SBUF layouts the kernel reads."""
    f16 = np.float16
    t = np.arange(BS)
    n0 = np.arange(128)
    ang = 2.0 * np.pi / BS

    CF0 = np.cos(ang * np.outer(t, n0))
    CF1 = np.empty((BS, 128))
    CF1[:, 0] = np.cos(np.pi * t)
    p = np.arange(1, 128)
    CF1[:, 1:] = -np.sin(ang * np.outer(t, p))
    CF0 = CF0[:, PERM]
    CF1 = CF1[:, PERM]
    cfs = np.stack([
        np.concatenate([CF0[:128], CF0[128:]], axis=1),
        np.concatenate([CF1[:128], CF1[128:]], axis=1),
    ], axis=1).astype(f16)                                  # [128, 2, 256]

    # wpk[(k*16+j), g, c, r'(i,k)] = M_c[i, j, 16k+g];  M = (Wr, Wi, -Wi);
    # r'(i,k) = 32*(i//4) + i%4 + 4k is the E-output row for (i, k)
    wpk = np.zeros((128, NG, 3, 128), dtype=f16)
    jj = np.arange(KIN)[:, None, None]
    ii = np.arange(KOUT)[None, :, None]
    kk = np.arange(8)[None, None, :]
    rr = 32 * (ii // 4) + ii % 4 + 4 * kk
    for g in range(NG):
        for c, M in enumerate((W_real, W_imag, -W_imag)):
            wpk[kk * 16 + jj, g, c, rr] = M[ii, jj, 16 * kk + g]
    # wnyq[j, r'(i,0)] = Wr[i, j, 128]: the Nyquist matmul output lands
    # directly on the E-output rows for k=0, 32-aligned for the copies
    wnyq = np.zeros((KIN, 128), dtype=f16)
    for i in range(KOUT):
        wnyq[:, 32 * (i // 4) + i % 4] = W_real[i, :, 128]

    # IDFT matrices in natural frequency row order (yh row p holds n = p)
    m = np.arange(BS)
    D0 = np.empty((128, BS))
    D0[0] = 1.0 / BS
    nn = np.arange(1, 128)
    D0[1:] = (2.0 / BS) * np.cos(ang * np.outer(nn, m))
    D1 = np.empty((128, BS))
    D1[0] = ((-1.0) ** m) / BS
    D1[1:] = -(2.0 / BS) * np.sin(ang * np.outer(nn, m))
    dmat = np.stack([D0, D1], axis=1).astype(f16)  # [128, 2, 256]

    ident = np.eye(128, dtype=np.float32)
    return {"cfs": cfs, "wpk": wpk, "wnyq": wnyq, "dmat": dmat, "ident": ident}


def _build_program():
    nc = bacc.Bacc(
        "TRN2", target_bir_lowering=False, debug=False, num_devices=NCORES
    )
    x_d = nc.dram_tensor("x", [BC, D_IN], F32, kind="ExternalInput").ap()
    cfs_d = nc.dram_tensor("cfs", [128, 2, 256], F16, kind="ExternalInput").ap()
    wpk_d = nc.dram_tensor("wpk", [128, NG, 3, 128], F16, kind="ExternalInput").ap()
    wnyq_d = nc.dram_tensor("wnyq", [KIN, 128], F16, kind="ExternalInput").ap()
    dmat_d = nc.dram_tensor("dmat", [128, 2, 256], F16, kind="ExternalInput").ap()
    ident_d = nc.dram_tensor("ident", [128, 128], F32, kind="ExternalInput").ap()
    out_d = nc.dram_tensor("out", [BC, D_OUT], F32, kind="ExternalOutput").ap()

    cp_state = [0]

    with tile.TileContext(nc) as tc, ExitStack() as ctx:
        def copy(dst, src):
            # alternate PSUM->SBUF copies between DVE and ACT
            if cp_state[0] % 2 == 0:
                nc.vector.tensor_copy(dst, src)
            else:
                nc.scalar.copy(dst, src)
            cp_state[0] += 1

        consts = ctx.enter_context(tc.tile_pool(name="consts", bufs=1))
        xsp = ctx.enter_context(tc.tile_pool(name="xsp", bufs=3))
        xtp = ctx.enter_context(tc.tile_pool(name="xtp", bufs=1))
        mid1 = ctx.enter_context(tc.tile_pool(name="mid1", bufs=1))
        mid2 = ctx.enter_context(tc.tile_pool(name="mid2", bufs=2))
        osp = ctx.enter_context(tc.tile_pool(name="osp", bufs=2))
        ps = ctx.enter_context(tc.tile_pool(name="ps", bufs=4, space="PSUM"))

        cfs = consts.tile([128, 2, 256], F16)
        wpk = consts.tile([128, NG, 3, 128], F16)
        wnyq = consts.tile([KIN, 128], F16)
        dmat = consts.tile([128, 2, 256], F16)
        ident = consts.tile([128, 128], F32)

        # ident/cfs (small, needed first) go ahead of x on the sync ring;
        # the bulky-but-late W/IDFT consts follow the x loads there, so
        # nothing contends with chunk 0 of x (first compute dependency)
        nc.sync.dma_start(ident[:], ident_d)
        nc.sync.dma_start(cfs[:], cfs_d)

        # ---- input loads: all on the sync ring, in order.  One ring's
        # engines drain its DMAs in issue order, so chunk 0 completes
        # first (~10us) instead of fair-sharing with later chunks.
        xs = [xsp.tile([128, D_IN], F32, tag="xs", name=f"xs{i}")
              for i in range(4)]
        for bc in range(4):
            for h in range(2):
                nc.sync.dma_start(
                    xs[bc][:, 2048 * h:2048 * (h + 1)],
                    x_d[128 * bc:128 * (bc + 1), 2048 * h:2048 * (h + 1)])
        nc.sync.dma_start(wpk[:], wpk_d)
        nc.sync.dma_start(dmat[:], dmat_d)
        nc.sync.dma_start(wnyq[:], wnyq_d)

        # per-slab tiles; gg/yh double-buffered so regroup DMAs of slab 1
        # don't wait on slab 0's consumers
        xt = [xtp.tile([128, 32, SLAB], F16, tag="xt", name=f"xt{i}")
              for i in range(2)]
        xf = [mid1.tile([128, KIN, 2, SLAB], F16, tag="xf", name=f"xf{i}")
              for i in range(2)]
        # gg/yh are split into halves so E/I can start once the first half
        # of a regroup has landed instead of waiting for all 16 DMAs
        gg = [[mid2.tile([128, NG // 2, 2, SLAB], F16, tag=f"gg{h}",
                         name=f"gg{i}_{h}") for h in range(2)]
              for i in range(2)]
        yy = [mid1.tile([128, NG, 2, SLAB], F16, tag="yy", name=f"yy{i}")
              for i in range(2)]
        yh = [[mid2.tile([128, KOUT // 2, 2, SLAB], F16, tag=f"yh{h}",
                         name=f"yh{i}_{h}") for h in range(2)]
              for i in range(2)]
        gnyq = [consts.tile([KIN, SLAB], F16, tag="gnyq", name=f"gnyq{i}")
                for i in range(2)]

        # regroup ring plans: regroup1(s0) avoids sync (still loading x);
        # later regroups spread across all three DGE units with the
        # otherwise-idle gpsimd ring taking the largest share
        RR_NOSYNC = [nc.gpsimd, nc.scalar] * 8
        RR_ALL = [nc.gpsimd, nc.sync, nc.gpsimd, nc.scalar] * 4

        def stage_T(s, c):
            # transpose chunk bc=2s+c of x into xt[s][:, :, 128c:128c+128]
            bc = 2 * s + c
            for dcg in range(4):
                pt = ps.tile([128, 8, 128], F32, tag="ps")
                for q in range(8):
                    dc = dcg * 8 + q
                    nc.tensor.transpose(
                        pt[:, q, :], xs[bc][:, 128 * dc:128 * (dc + 1)],
                        ident[:],
                    )
                copy(xt[s][:, 8 * dcg:8 * dcg + 8, 128 * c:128 * (c + 1)],
                     pt[:])

        def stage_F(s):
            # real DFT: xf[s][n, j, which, b] = sum_t cfs[t, which, n] xt[t, (j,tc), b]
            # j pairs map to the two banks of one PSUM slot (slots 0 / 2) so
            # accumulation groups never interleave within a bank, while each
            # LDWEIGHTS (cfs half) feeds two matmuls.
            # slot map: j0->0(bank0), j1->2(bank1), then j2->1(bank0), j3->3
            # (a bank's second group starts only after its first stopped)
            for which in range(2):
                for jg in range(4):
                    pf = ps.tile([128, 4, 256], F32, tag="ps")
                    for half in range(2):
                        for tc_ in range(2):
                            for bank in range(2):
                                j = 4 * jg + 2 * half + bank
                                nc.tensor.matmul(
                                    pf[:, 2 * bank + half, :],
                                    cfs[:, which, 128 * tc_:128 * (tc_ + 1)],
                                    xt[s][:, 2 * j + tc_, :],
                                    start=(tc_ == 0),
                                    stop=(tc_ == 1),
                                )
                    # slots (0,2,1,3) hold j order (0,1,2,3)
                    copy(xf[s][:, 4 * jg:4 * jg + 4, which, :],
                         pf[:].rearrange("p (a b) n -> p b a n", a=2))

        def regroup1(s, rings):
            # gg[s][(k,j), g, ri, b] = xf[s][r(k,g), j, ri, b]; one DMA per
            # g reading a stride-4 partition slice (8 SBUF ports)
            for g in range(NG):
                a = _grp_rows(g)
                rings[g % len(rings)].dma_start(
                    out=gg[s][g // 8][:, g % 8, :, :],
                    in_=xf[s][a:a + 29:4, :, :, :])

        def stage_E(s):
            # per-frequency-group complex mixing, two g per PSUM slot:
            # g even -> slots 0 (Yr) / 2 (Yi), g odd -> slots 1 / 3
            for gp in range(8):
                pe = ps.tile([128, 4, 256], F32, tag="ps")
                for half in range(2):
                    g = 2 * gp + half
                    nc.tensor.matmul(pe[:, half, :], wpk[:, g, 0, :],
                                     gg[s][g // 8][:, g % 8, 0, :], start=True, stop=False)
                    nc.tensor.matmul(pe[:, 2 + half, :], wpk[:, g, 0, :],
                                     gg[s][g // 8][:, g % 8, 1, :], start=True, stop=False)
                    nc.tensor.matmul(pe[:, half, :], wpk[:, g, 1, :],
                                     gg[s][g // 8][:, g % 8, 1, :], start=False, stop=True)
                    nc.tensor.matmul(pe[:, 2 + half, :], wpk[:, g, 2, :],
                                     gg[s][g // 8][:, g % 8, 0, :], start=False, stop=True)
                # slots (0,2,1,3) hold (g0 Yr, g0 Yi, g1 Yr, g1 Yi)
                copy(yy[s][:, 2 * gp:2 * gp + 2, :, :],
                     pe[:].rearrange("p (a b) n -> p b a n", a=2))
            # Nyquist einsum lands in the (f=0,g=0) rows of the imag half
            # (the otherwise meaningless Zi[0] slots); regroup2 then routes
            # it to yh[.,1,...] row 0, where dmat row 0 of D1 applies it.
            pyn = ps.tile([128, 256], F32, tag="ps")
            nc.tensor.matmul(pyn[:], wnyq[:], gnyq[s][:],
                             start=True, stop=True)
            # Zi[0] rows for i = 4a+c sit at partition 32a+c; pyn rows
            # match, so each copy reads/writes a 32-aligned partition base
            for a in range(4):
                copy(yy[s][32 * a:32 * a + 4, 0, 1, :],
                     pyn[32 * a:32 * a + 4, :])

        def regroup2(s, rings):
            # yh[s][n, i, ri, b] = yy[s][r'(i,k), g, ri, b]; one DMA per i
            # reading a stride-4 partition slice; yh rows come out in
            # natural frequency order n = 16k+g
            for i in range(KOUT):
                a = _grp_rows(i)
                rings[i % len(rings)].dma_start(
                    out=yh[s][i // 8][:, i % 8, :, :],
                    in_=yy[s][a:a + 29:4, :, :, :])

        def stage_I(s):
            # inverse DFT with the data stationary -> [b, m] orientation;
            # two i per PSUM slot: i even -> slots 0 (c=0) / 2 (c=1),
            # i odd -> slots 1 / 3
            for ig in range(4):
                osb = osp.tile([128, 2, 1024], F32, tag="os")
                for ip in range(2):
                    po = ps.tile([128, 4, 256], F32, tag="ps")
                    for half in range(2):
                        i = 4 * ig + 2 * ip + half
                        for c in range(2):   # bs chunk -> banks 0 / 1
                            nc.tensor.matmul(
                                po[:, 2 * c + half, :],
                                yh[s][i // 8][:, i % 8, 0,
                                              128 * c:128 * (c + 1)],
                                dmat[:, 0, :], start=True, stop=False)
                        for c in range(2):
                            nc.tensor.matmul(
                                po[:, 2 * c + half, :],
                                yh[s][i // 8][:, i % 8, 1,
                                              128 * c:128 * (c + 1)],
                                dmat[:, 1, :], start=False, stop=True)
                    # slots (0,1,2,3) = (i0c0, i1c0, i0c1, i1c1) = dst order
                    copy(osb[:, :, 512 * ip:512 * (ip + 1)], po[:])
                # store [256 rows, 1024 cols] of out, alternating rings
                eng = nc.sync if (4 * s + ig) % 2 == 0 else nc.scalar
                eng.dma_start(
                    out_d[SLAB * s:SLAB * (s + 1),
                          1024 * ig:1024 * (ig + 1)].rearrange(
                              "(c p) m -> p c m", c=2),
                    osb[:],
                )

        # ---- pipelined emission (per-engine queues in execution order)
        # PE warmup: dummy transposes of ident bridge the input-load
        # window so HAM unthrottles before the real work starts
        for w in range(2):
            pw = ps.tile([128, 4, 128], F32, tag="ps", name=f"pw{w}")
            for q in range(4):
                nc.tensor.transpose(pw[:, q, :], ident[:], ident[:])

        stage_T(0, 0)
        stage_T(0, 1)
        stage_F(0)
        regroup1(0, RR_NOSYNC)
        nc.gpsimd.dma_start(out=gnyq[0][:], in_=xf[0][0:1, :, 1, :])
        stage_T(1, 0)
        stage_T(1, 1)
        stage_F(1)
        regroup1(1, RR_ALL)
        nc.gpsimd.dma_start(out=gnyq[1][:], in_=xf[1][0:1, :, 1, :])
        stage_E(0)
        regroup2(0, RR_ALL)
        stage_E(1)
        regroup2(1, RR_ALL)
        stage_I(0)
        stage_I(1)

    nc.compile()
    return nc


def _get_program():
    if "nc" not in _CACHE:
        _CACHE["nc"] = _build_program()
    return _CACHE["nc"]


def _install_ntff_hook():
    """Provide antenv.axon_hooks (absent in this image) so that
    run_bass_kernel_spmd(trace=True) can capture NTFF profiles through the
    axon client library."""
    import sys
    import types
    import ctypes
    import contextlib

    if "antenv.axon_hooks" in sys.modules:
        return
    try:
        lib = ctypes.CDLL("/opt/axon/libaxon_pjrt.so")
    except OSError:
        return
    if not hasattr(lib, "axon_start_nrt_profile"):
        return
    lib.axon_start_nrt_profile.argtypes = [
        ctypes.POINTER(ctypes.c_int64),
        ctypes.c_size_t,
    ]
    lib.axon_start_nrt_profile.restype = ctypes.c_int64
    lib.axon_stop_nrt_profile.argtypes = [ctypes.c_char_p]
    lib.axon_stop_nrt_profile.restype = ctypes.c_int64

    @contextlib.contextmanager
    def _hook(output_dir, device_ids):
        import jax

        jax.devices()
        if device_ids:
            ids = (ctypes.c_int64 * len(device_ids))(*device_ids)
            rc = lib.axon_start_nrt_profile(ids, len(device_ids))
        else:
            rc = lib.axon_start_nrt_profile(None, 0)
        if rc != 0:
            raise RuntimeError(f"axon_start_nrt_profile rc={rc}")
        try:
            yield
        finally:
            n = lib.axon_stop_nrt_profile(str(output_dir).encode())
            print(f"ntff profile: {n} file(s) -> {output_dir}")

    mod = types.ModuleType("antenv.axon_hooks")
    state = {"hook": _hook}
    mod.get_axon_ntff_profile_hook = lambda: state["hook"]
    mod.set_axon_ntff_profile_hook = lambda h: state.update(hook=h)
    sys.modules["antenv.axon_hooks"] = mod
    import antenv

    antenv.axon_hooks = mod


def kernel(x, W_real, W_imag, block_size, out_features):
    global LAST_RESULTS
    x = np.ascontiguousarray(np.asarray(x, dtype=np.float32))
    Wr = np.asarray(W_real, dtype=np.float32)
    Wi = np.asarray(W_imag, dtype=np.float32)
    assert int(block_size) == BS and int(out_features) == D_OUT
    assert x.shape == (B_FULL, D_IN) and Wr.shape == (KOUT, KIN, 129)

    nc = _get_program()
    consts = _build_consts(Wr, Wi)
    core_ids = list(range(NCORES))
    in_maps = [
        {"x": np.ascontiguousarray(x[c * BC:(c + 1) * BC]), **consts}
        for c in core_ids
    ]
    trace = bool(int(os.environ.get("KERNEL_TRACE", "0")))
    if trace:
        _install_ntff_hook()
    res = run_bass_kernel_spmd(nc, in_maps, core_ids, trace=trace)
    LAST_RESULTS = res
    out = np.concatenate([res.results[c]["out"] for c in core_ids], axis=0)
    return np.ascontiguousarray(out.astype(np.float32))
